# revision 6
# baseline (speedup 1.0000x reference)
"""Trainium2 Bass kernel for ContrastiveLoss (N=16384, D=1024, 8 NeuronCores).

Strategy (data-parallel over anchors):
  - Host shards rows across 8 cores: core i owns anchor rows [2048*i, 2048*(i+1)).
  - Host gathers pos/neg rows (gather commutes with row-wise normalization),
    so each core receives three contiguous [2048, 1024] f32 blocks.
  - Device computes, per row r: sum(u*u), sum(u*v), sum(u*w) with a
    double-buffered raw-Bass pipeline:
      ScalarE: Square+accum (row norm^2), Copy+accum (reduce of u*v product)
      VectorE: tensor_tensor mult (u*v, u*w), tensor_reduce (u*w product)
      SP:      2MB HWDGE DMA loads, stats store
  - Row norms of pos/neg rows are gathers of the same global norm array, so
    the host epilogue (f64) reconstructs the reference math exactly:
      ||a-b+eps||^2 = |a|^2 + |b|^2 + D*eps^2 - 2<a,b> (+ O(eps) sum terms,
      dropped: ~1e-8 relative), a = u/max(|u|,eps), then the margin loss.
"""

import sys

for _p in ("/opt/trn_rl_repo", "/root/.axon_site/_ro/trn_rl_repo"):
    if _p not in sys.path:
        sys.path.append(_p)

import numpy as np

N = 16384  # total rows
D = 1024  # embedding dim
NCORES = 8
RPC = N // NCORES  # rows per core = 2048
T = RPC // 128  # row-tiles per core = 16
G = 4  # row-tiles per DMA group (2 MB per load)
NG = T // G  # DMA groups per core = 4
EPS = 1e-6
MARGIN = 1.0

LAST_RESULT = None
_CACHE = {}


def _build_nc():
    import concourse.bass as bass
    import concourse.mybir as mybir

    f32 = mybir.dt.float32
    nc = bass.Bass()
    anc = nc.declare_dram_parameter("anc", [RPC, D], f32, isOutput=False)
    pos = nc.declare_dram_parameter("pos", [RPC, D], f32, isOutput=False)
    neg = nc.declare_dram_parameter("neg", [RPC, D], f32, isOutput=False)
    out = nc.declare_dram_parameter("out", [3, 128, T], f32, isOutput=True)

    # DRAM row-tile t holds rows [128*t, 128*t+128); G tiles per DMA group.
    anc_r = anc[:, :].rearrange("(g a p) d -> g p a d", p=128, a=G)
    pos_r = pos[:, :].rearrange("(g a p) d -> g p a d", p=128, a=G)
    neg_r = neg[:, :].rearrange("(g a p) d -> g p a d", p=128, a=G)
    out_ap = out[:, :, :]

    Sq = mybir.ActivationFunctionType.Square
    Cp = mybir.ActivationFunctionType.Copy
    mult = mybir.AluOpType.mult
    add = mybir.AluOpType.add
    AX = mybir.AxisListType.X

    from contextlib import ExitStack

    with ExitStack() as ctx:
        u0 = ctx.enter_context(nc.sbuf_tensor([128, G, D], f32))
        u1 = ctx.enter_context(nc.sbuf_tensor([128, G, D], f32))
        v0 = ctx.enter_context(nc.sbuf_tensor([128, G, D], f32))
        v1 = ctx.enter_context(nc.sbuf_tensor([128, G, D], f32))
        w0 = ctx.enter_context(nc.sbuf_tensor([128, G, D], f32))
        w1 = ctx.enter_context(nc.sbuf_tensor([128, G, D], f32))
        sqd0 = ctx.enter_context(nc.sbuf_tensor([128, D], f32))  # ACT Square dumps
        sqd1 = ctx.enter_context(nc.sbuf_tensor([128, D], f32))
        cpd0 = ctx.enter_context(nc.sbuf_tensor([128, D], f32))  # ACT Copy dumps
        cpd1 = ctx.enter_context(nc.sbuf_tensor([128, D], f32))
        s2a = ctx.enter_context(nc.sbuf_tensor([128, D], f32))  # DVE->ACT product
        s2b = ctx.enter_context(nc.sbuf_tensor([128, D], f32))
        s3a = ctx.enter_context(nc.sbuf_tensor([128, D], f32))  # DVE-local scratch
        s3b = ctx.enter_context(nc.sbuf_tensor([128, D], f32))
        nu2 = ctx.enter_context(nc.sbuf_tensor([128, T], f32))
        dotp = ctx.enter_context(nc.sbuf_tensor([128, T], f32))
        dotn = ctx.enter_context(nc.sbuf_tensor([128, T], f32))
        # per-(tensor, parity) load sems: at most one outstanding DMA each,
        # so completion order is unambiguous
        sem_u0 = ctx.enter_context(nc.semaphore())
        sem_u1 = ctx.enter_context(nc.semaphore())
        sem_v0 = ctx.enter_context(nc.semaphore())
        sem_v1 = ctx.enter_context(nc.semaphore())
        sem_w0 = ctx.enter_context(nc.semaphore())
        sem_w1 = ctx.enter_context(nc.semaphore())
        st_sem = ctx.enter_context(nc.semaphore())  # +16 per completed store DMA
        # per-op-class retirement sems (count = sub-tiles retired); these give
        # the race detector an explicit edge for every buffer reuse
        dve_s2 = ctx.enter_context(nc.semaphore())  # TT#1 (u*v -> S2)
        dve_s3 = ctx.enter_context(nc.semaphore())  # TT#2 (u*w -> s3)
        dve_red = ctx.enter_context(nc.semaphore())  # reduce (s3 -> dotn col)
        act_sq = ctx.enter_context(nc.semaphore())  # Square (u -> nu2 col)
        act_s2 = ctx.enter_context(nc.semaphore())  # Copy (S2 -> dotp col)
        block = ctx.enter_context(nc.Block())

        U = [u0, u1]
        V = [v0, v1]
        W = [w0, w1]
        S2 = [s2a, s2b]
        S3 = [s3a, s3b]
        SQD = [sqd0, sqd1]
        CPD = [cpd0, cpd1]
        SEM_U = [sem_u0, sem_u1]
        SEM_V = [sem_v0, sem_v1]
        SEM_W = [sem_w0, sem_w1]

        @block.sync
        def _(sync):
            for g in range(NG):
                if g >= 2:
                    # consumers of buffer set g-2 must have retired
                    sync.wait_ge(dve_s2, G * (g - 1))  # TT#1 reads of U,V
                    sync.wait_ge(dve_s3, G * (g - 1))  # TT#2 reads of U,W
                    sync.wait_ge(act_sq, G * (g - 1))  # Square reads of U
                b = g % 2
                sync.dma_start(out=U[b][:], in_=anc_r[g]).then_inc(SEM_U[b], 16)
                sync.dma_start(out=V[b][:], in_=pos_r[g]).then_inc(SEM_V[b], 16)
                sync.dma_start(out=W[b][:], in_=neg_r[g]).then_inc(SEM_W[b], 16)
            sync.wait_ge(act_sq, T)
            sync.wait_ge(act_s2, T)
            sync.wait_ge(dve_red, T)
            sync.dma_start(out=out_ap[0], in_=nu2[:]).then_inc(st_sem, 16)
            sync.dma_start(out=out_ap[1], in_=dotp[:]).then_inc(st_sem, 16)
            sync.dma_start(out=out_ap[2], in_=dotn[:]).then_inc(st_sem, 16)
            sync.wait_ge(st_sem, 48)

        @block.vector
        def _(vector):
            for g in range(NG):
                b = g % 2
                k = 16 * (g // 2 + 1)
                vector.wait_ge(SEM_U[b], k)
                vector.wait_ge(SEM_V[b], k)
                for a in range(G):
                    t = g * G + a
                    j = t % 2
                    us = U[b][:, a, :]
                    vs = V[b][:, a, :]
                    ws = W[b][:, a, :]
                    if t >= 2:
                        vector.wait_ge(act_s2, t - 1)  # S2[j] consumed
                    nc.vector.tensor_tensor(
                        out=S2[j][:], in0=us, in1=vs, op=mult
                    ).then_inc(dve_s2, 1)
                    if a == 0:
                        vector.wait_ge(SEM_W[b], k)  # w loaded
                    if t >= 2:
                        vector.wait_ge(dve_red, t - 1)  # S3[j] reduce retired
                    nc.vector.tensor_tensor(
                        out=S3[j][:], in0=us, in1=ws, op=mult
                    ).then_inc(dve_s3, 1)
                    vector.wait_ge(dve_s3, t + 1)  # same-engine RAW on S3[j]
                    nc.vector.tensor_reduce(
                        out=dotn[:, t : t + 1], in_=S3[j][:], axis=AX, op=add
                    ).then_inc(dve_red, 1)

        @block.scalar
        def _(scalar):
            for g in range(NG):
                b = g % 2
                scalar.wait_ge(SEM_U[b], 16 * (g // 2 + 1))  # u of group g loaded
                for a in range(G):
                    t = g * G + a
                    j = t % 2
                    us = U[b][:, a, :]
                    if t >= 2:
                        scalar.wait_ge(act_sq, t - 1)  # SQD[j] writer retired
                    nc.scalar.activation(
                        out=SQD[j][:], in_=us, func=Sq, accum_out=nu2[:, t : t + 1]
                    ).then_inc(act_sq, 1)
                    scalar.wait_ge(dve_s2, t + 1)  # product ready
                    if t >= 2:
                        scalar.wait_ge(act_s2, t - 1)  # CPD[j] writer retired
                    nc.scalar.activation(
                        out=CPD[j][:], in_=S2[j][:], func=Cp, accum_out=dotp[:, t : t + 1]
                    ).then_inc(act_s2, 1)

    return nc


def kernel(embeddings, labels, pos_idx, neg_idx):
    global LAST_RESULT
    from concourse.bass_utils import run_bass_kernel_spmd

    emb = np.ascontiguousarray(np.asarray(embeddings, dtype=np.float32))
    assert emb.shape == (N, D)
    pidx = np.asarray(pos_idx).astype(np.int64)
    nidx = np.asarray(neg_idx).astype(np.int64)

    in_maps = []
    for i in range(NCORES):
        sl = slice(i * RPC, (i + 1) * RPC)
        in_maps.append(
            {
                "anc": emb[sl],
                "pos": np.ascontiguousarray(emb[pidx[sl]]),
                "neg": np.ascontiguousarray(emb[nidx[sl]]),
            }
        )

    nc = _CACHE.get("nc")
    if nc is None:
        nc = _build_nc()
        _CACHE["nc"] = nc

    res = run_bass_kernel_spmd(nc, in_maps, list(range(NCORES)))
    LAST_RESULT = res

    # out[k] is [128, T]: row p, col t -> shard row t*128+p
    def decode(k):
        return np.concatenate(
            [res.results[i]["out"][k].T.ravel() for i in range(NCORES)]
        ).astype(np.float64)

    nu2 = decode(0)
    P = decode(1)
    Q = decode(2)

    norm = np.sqrt(nu2)
    den = np.maximum(norm, EPS)  # F.normalize clamp
    ahat2 = nu2 / (den * den)  # ||a_hat||^2 (==1 unless degenerate)

    def dist(idx, dot):
        S = ahat2 + ahat2[idx] - 2.0 * dot / (den * den[idx]) + D * EPS * EPS
        return np.sqrt(np.maximum(S, 0.0)) + EPS

    d_pos = dist(pidx, P)
    d_neg = dist(nidx, Q)
    pos_loss = d_pos * d_pos
    neg_loss = np.maximum(MARGIN - d_neg, EPS) ** 2
    total = pos_loss.sum() + neg_loss.sum()
    return np.array(total / (2.0 * N), dtype=np.float32)


# revision 8
# speedup vs baseline: 1.3939x; 1.3939x over previous
"""Trainium2 Bass kernel for ContrastiveLoss (N=16384, D=1024, 8 NeuronCores).

Strategy (data-parallel over anchors):
  - Host shards rows across 8 cores: core i owns anchor rows [2048*i, 2048*(i+1)).
  - Host gathers pos/neg rows (gather commutes with row-wise normalization) and
    converts to fp16, so each core receives three contiguous [2048, 1024] fp16
    blocks (halves HBM traffic; fp16 keeps ~1e-5 relative accuracy here).
  - Device computes, per row r: sum(u*u), sum(u*v), sum(u*w) with a
    triple-buffered raw-Bass pipeline:
      ScalarE: Square+accum (row norm^2), Copy+accum (reduce of u*v product)
      VectorE: tensor_tensor mult fp16 2x mode (u*v, u*w), tensor_reduce (u*w)
      SP:      1MB HWDGE DMA loads, stats store
  - Row norms of pos/neg rows are gathers of the same global norm array, so
    the host epilogue (f64) reconstructs the reference math:
      ||a-b+eps||^2 = |a|^2 + |b|^2 + D*eps^2 - 2<a,b> (+ O(eps) sum terms,
      dropped: ~1e-8 relative), a = u/max(|u|,eps), then the margin loss.
"""

import sys

for _p in ("/opt/trn_rl_repo", "/root/.axon_site/_ro/trn_rl_repo"):
    if _p not in sys.path:
        sys.path.append(_p)

import numpy as np

N = 16384  # total rows
D = 1024  # embedding dim
NCORES = 8
RPC = N // NCORES  # rows per core = 2048
T = RPC // 128  # row-tiles per core = 16
G = 4  # row-tiles per DMA group (1 MB fp16 per load)
NG = T // G  # DMA groups per core = 4
BUFS = 3  # in-flight groups
EPS = 1e-6
MARGIN = 1.0

LAST_RESULT = None
_CACHE = {}


def _build_nc():
    import concourse.bass as bass
    import concourse.mybir as mybir

    f32 = mybir.dt.float32
    f16 = mybir.dt.float16
    nc = bass.Bass()
    anc = nc.declare_dram_parameter("anc", [RPC, D], f16, isOutput=False)
    pos = nc.declare_dram_parameter("pos", [RPC, D], f16, isOutput=False)
    neg = nc.declare_dram_parameter("neg", [RPC, D], f16, isOutput=False)
    out = nc.declare_dram_parameter("out", [3, 128, T], f32, isOutput=True)

    # DRAM row-tile t holds rows [128*t, 128*t+128); G tiles per DMA group.
    anc_r = anc[:, :].rearrange("(g a p) d -> g p a d", p=128, a=G)
    pos_r = pos[:, :].rearrange("(g a p) d -> g p a d", p=128, a=G)
    neg_r = neg[:, :].rearrange("(g a p) d -> g p a d", p=128, a=G)
    out_ap = out[:, :, :]

    Sq = mybir.ActivationFunctionType.Square
    Cp = mybir.ActivationFunctionType.Copy
    mult = mybir.AluOpType.mult
    add = mybir.AluOpType.add
    AX = mybir.AxisListType.X

    from contextlib import ExitStack

    with ExitStack() as ctx:
        sb = lambda nm, shape, dt: ctx.enter_context(nc.sbuf_tensor(nm, shape, dt))
        sem = lambda nm: ctx.enter_context(nc.semaphore(nm))

        U = [sb(f"u{i}", [128, G, D], f16) for i in range(BUFS)]
        V = [sb(f"v{i}", [128, G, D], f16) for i in range(BUFS)]
        W = [sb(f"w{i}", [128, G, D], f16) for i in range(BUFS)]
        SQD = [sb(f"sqd{i}", [128, D], f16) for i in range(2)]  # ACT Square dumps
        CPD = [sb(f"cpd{i}", [128, D], f16) for i in range(2)]  # ACT Copy dumps
        S2 = [sb(f"s2{i}", [128, D], f16) for i in range(2)]  # DVE->ACT u*v product
        S3 = [sb(f"s3{i}", [128, D], f16) for i in range(2)]  # DVE-local u*w product
        nu2 = sb("nu2", [128, T], f32)
        dotp = sb("dotp", [128, T], f32)
        dotn = sb("dotn", [128, T], f32)
        # per-(tensor, slot) load sems: at most one outstanding DMA each,
        # so completion order is unambiguous
        SEM_U = [sem(f"sem_u{i}") for i in range(BUFS)]
        SEM_V = [sem(f"sem_v{i}") for i in range(BUFS)]
        SEM_W = [sem(f"sem_w{i}") for i in range(BUFS)]
        st_sem = sem("st_sem")  # +16 per completed store DMA
        # per-op-class retirement sems (count = sub-tiles retired); these give
        # the race detector an explicit edge for every buffer reuse
        dve_s2 = sem("dve_s2")  # TT#1 (u*v -> S2)
        dve_s3 = sem("dve_s3")  # TT#2 (u*w -> S3)
        dve_red = sem("dve_red")  # reduce (S3 -> dotn col)
        act_sq = sem("act_sq")  # Square (u -> nu2 col)
        act_s2 = sem("act_s2")  # Copy (S2 -> dotp col)
        block = ctx.enter_context(nc.Block())

        @block.sync
        def _(sync):
            for g in range(NG):
                if g >= BUFS:
                    # consumers of buffer set g-BUFS must have retired
                    sync.wait_ge(dve_s2, G * (g - BUFS + 1))  # TT#1 reads U,V
                    sync.wait_ge(dve_s3, G * (g - BUFS + 1))  # TT#2 reads U,W
                    sync.wait_ge(act_sq, G * (g - BUFS + 1))  # Square reads U
                b = g % BUFS
                sync.dma_start(out=U[b][:], in_=anc_r[g]).then_inc(SEM_U[b], 16)
                sync.dma_start(out=V[b][:], in_=pos_r[g]).then_inc(SEM_V[b], 16)
                sync.dma_start(out=W[b][:], in_=neg_r[g]).then_inc(SEM_W[b], 16)
            sync.wait_ge(act_sq, T)
            sync.wait_ge(act_s2, T)
            sync.wait_ge(dve_red, T)
            sync.dma_start(out=out_ap[0], in_=nu2[:]).then_inc(st_sem, 16)
            sync.dma_start(out=out_ap[1], in_=dotp[:]).then_inc(st_sem, 16)
            sync.dma_start(out=out_ap[2], in_=dotn[:]).then_inc(st_sem, 16)
            sync.wait_ge(st_sem, 48)

        @block.vector
        def _(vector):
            for g in range(NG):
                b = g % BUFS
                k = 16 * (g // BUFS + 1)
                vector.wait_ge(SEM_U[b], k)
                vector.wait_ge(SEM_V[b], k)
                for a in range(G):
                    t = g * G + a
                    j = t % 2
                    us = U[b][:, a, :]
                    vs = V[b][:, a, :]
                    ws = W[b][:, a, :]
                    if t >= 2:
                        vector.wait_ge(act_s2, t - 1)  # S2[j] consumed
                    nc.vector.tensor_tensor(
                        out=S2[j][:], in0=us, in1=vs, op=mult
                    ).then_inc(dve_s2, 1)
                    if a == 0:
                        vector.wait_ge(SEM_W[b], k)  # w loaded
                    if t >= 2:
                        vector.wait_ge(dve_red, t - 1)  # S3[j] reduce retired
                    nc.vector.tensor_tensor(
                        out=S3[j][:], in0=us, in1=ws, op=mult
                    ).then_inc(dve_s3, 1)
                    vector.wait_ge(dve_s3, t + 1)  # same-engine RAW on S3[j]
                    nc.vector.tensor_reduce(
                        out=dotn[:, t : t + 1], in_=S3[j][:], axis=AX, op=add
                    ).then_inc(dve_red, 1)

        @block.scalar
        def _(scalar):
            for g in range(NG):
                b = g % BUFS
                scalar.wait_ge(SEM_U[b], 16 * (g // BUFS + 1))  # u loaded
                for a in range(G):
                    t = g * G + a
                    j = t % 2
                    us = U[b][:, a, :]
                    if t >= 2:
                        scalar.wait_ge(act_sq, t - 1)  # SQD[j] writer retired
                    nc.scalar.activation(
                        out=SQD[j][:], in_=us, func=Sq, accum_out=nu2[:, t : t + 1]
                    ).then_inc(act_sq, 1)
                    scalar.wait_ge(dve_s2, t + 1)  # product ready
                    if t >= 2:
                        scalar.wait_ge(act_s2, t - 1)  # CPD[j] writer retired
                    nc.scalar.activation(
                        out=CPD[j][:], in_=S2[j][:], func=Cp, accum_out=dotp[:, t : t + 1]
                    ).then_inc(act_s2, 1)

    return nc


def kernel(embeddings, labels, pos_idx, neg_idx):
    global LAST_RESULT
    from concourse.bass_utils import run_bass_kernel_spmd

    emb = np.asarray(embeddings, dtype=np.float32).astype(np.float16)
    assert emb.shape == (N, D)
    pidx = np.asarray(pos_idx).astype(np.int64)
    nidx = np.asarray(neg_idx).astype(np.int64)

    in_maps = []
    for i in range(NCORES):
        sl = slice(i * RPC, (i + 1) * RPC)
        in_maps.append(
            {
                "anc": np.ascontiguousarray(emb[sl]),
                "pos": np.ascontiguousarray(emb[pidx[sl]]),
                "neg": np.ascontiguousarray(emb[nidx[sl]]),
            }
        )

    nc = _CACHE.get("nc")
    if nc is None:
        nc = _build_nc()
        _CACHE["nc"] = nc

    res = run_bass_kernel_spmd(nc, in_maps, list(range(NCORES)))
    LAST_RESULT = res

    # out[k] is [128, T]: row p, col t -> shard row t*128+p
    def decode(k):
        return np.concatenate(
            [res.results[i]["out"][k].T.ravel() for i in range(NCORES)]
        ).astype(np.float64)

    nu2 = decode(0)
    P = decode(1)
    Q = decode(2)

    norm = np.sqrt(nu2)
    den = np.maximum(norm, EPS)  # F.normalize clamp
    ahat2 = nu2 / (den * den)  # ||a_hat||^2 (==1 unless degenerate)

    def dist(idx, dot):
        S = ahat2 + ahat2[idx] - 2.0 * dot / (den * den[idx]) + D * EPS * EPS
        return np.sqrt(np.maximum(S, 0.0)) + EPS

    d_pos = dist(pidx, P)
    d_neg = dist(nidx, Q)
    pos_loss = d_pos * d_pos
    neg_loss = np.maximum(MARGIN - d_neg, EPS) ** 2
    total = pos_loss.sum() + neg_loss.sum()
    return np.array(total / (2.0 * N), dtype=np.float32)


# revision 9
# speedup vs baseline: 1.5106x; 1.0837x over previous
"""Trainium2 Bass kernel for ContrastiveLoss (N=16384, D=1024, 8 NeuronCores).

Strategy (data-parallel over anchors):
  - Host shards rows across 8 cores: core i owns anchor rows [2048*i, 2048*(i+1)).
  - Host gathers pos/neg rows (gather commutes with row-wise normalization) and
    converts to fp16, so each core receives three contiguous [2048, 1024] fp16
    blocks (halves HBM traffic; fp16 keeps ~1e-5 relative accuracy here).
  - Device computes, per row r: sum(u*u), sum(u*v), sum(u*w) with a
    triple-buffered raw-Bass pipeline:
      ScalarE: Square+accum (row norm^2), Copy+accum (reduce of u*v product)
      VectorE: tensor_tensor mult fp16 2x mode (u*v, u*w), tensor_reduce (u*w)
      SP:      1MB HWDGE DMA loads, stats store
  - Row norms of pos/neg rows are gathers of the same global norm array, so
    the host epilogue (f64) reconstructs the reference math:
      ||a-b+eps||^2 = |a|^2 + |b|^2 + D*eps^2 - 2<a,b> (+ O(eps) sum terms,
      dropped: ~1e-8 relative), a = u/max(|u|,eps), then the margin loss.
"""

import sys

for _p in ("/opt/trn_rl_repo", "/root/.axon_site/_ro/trn_rl_repo"):
    if _p not in sys.path:
        sys.path.append(_p)

import numpy as np

N = 16384  # total rows
D = 1024  # embedding dim
NCORES = 8
RPC = N // NCORES  # rows per core = 2048
T = RPC // 128  # row-tiles per core = 16
G = 4  # row-tiles per DMA group (1 MB fp16 per load)
NG = T // G  # DMA groups per core = 4
BUFS = 3  # in-flight groups
EPS = 1e-6
MARGIN = 1.0

LAST_RESULT = None
_CACHE = {}


def _build_nc():
    import concourse.bass as bass
    import concourse.mybir as mybir

    f32 = mybir.dt.float32
    f16 = mybir.dt.float16
    nc = bass.Bass()
    anc = nc.declare_dram_parameter("anc", [RPC, D], f16, isOutput=False)
    pos = nc.declare_dram_parameter("pos", [RPC, D], f16, isOutput=False)
    neg = nc.declare_dram_parameter("neg", [RPC, D], f16, isOutput=False)
    out = nc.declare_dram_parameter("out", [3, 128, T], f32, isOutput=True)

    # DRAM row-tile t holds rows [128*t, 128*t+128); G tiles per DMA group.
    anc_r = anc[:, :].rearrange("(g a p) d -> g p a d", p=128, a=G)
    pos_r = pos[:, :].rearrange("(g a p) d -> g p a d", p=128, a=G)
    neg_r = neg[:, :].rearrange("(g a p) d -> g p a d", p=128, a=G)
    out_ap = out[:, :, :]

    Sq = mybir.ActivationFunctionType.Square
    Cp = mybir.ActivationFunctionType.Copy
    mult = mybir.AluOpType.mult
    add = mybir.AluOpType.add
    AX = mybir.AxisListType.X

    from contextlib import ExitStack

    with ExitStack() as ctx:
        sb = lambda nm, shape, dt: ctx.enter_context(nc.sbuf_tensor(nm, shape, dt))
        sem = lambda nm: ctx.enter_context(nc.semaphore(nm))

        U = [sb(f"u{i}", [128, G, D], f16) for i in range(BUFS)]
        V = [sb(f"v{i}", [128, G, D], f16) for i in range(BUFS)]
        W = [sb(f"w{i}", [128, G, D], f16) for i in range(BUFS)]
        ps = lambda nm, shape, dt: ctx.enter_context(nc.psum_tensor(nm, shape, dt))
        SQD = [ps(f"sqd{i}", [128, D], f32) for i in range(2)]  # ACT Square dumps
        CPD = [ps(f"cpd{i}", [128, D], f32) for i in range(2)]  # ACT Copy dumps
        S2 = [sb(f"s2{i}", [128, D], f16) for i in range(3)]  # DVE->ACT u*v product
        S3 = [sb(f"s3{i}", [128, D], f16) for i in range(3)]  # DVE-local u*w product
        nu2 = sb("nu2", [128, T], f32)
        dotp = sb("dotp", [128, T], f32)
        dotn = sb("dotn", [128, T], f32)
        # per-(tensor, slot) load sems: at most one outstanding DMA each,
        # so completion order is unambiguous
        SEM_U = [sem(f"sem_u{i}") for i in range(BUFS)]
        SEM_V = [sem(f"sem_v{i}") for i in range(BUFS)]
        SEM_W = [sem(f"sem_w{i}") for i in range(BUFS)]
        st_sem = sem("st_sem")  # +16 per completed store DMA
        # per-op-class retirement sems (count = sub-tiles retired); these give
        # the race detector an explicit edge for every buffer reuse
        dve_s2 = sem("dve_s2")  # TT#1 (u*v -> S2)
        dve_s3 = sem("dve_s3")  # TT#2 (u*w -> S3)
        dve_red = sem("dve_red")  # reduce (S3 -> dotn col)
        act_sq = sem("act_sq")  # Square (u -> nu2 col)
        act_s2 = sem("act_s2")  # Copy (S2 -> dotp col)
        block = ctx.enter_context(nc.Block())

        @block.sync
        def _(sync):
            for g in range(NG):
                if g >= BUFS:
                    # consumers of buffer set g-BUFS must have retired
                    sync.wait_ge(dve_s2, G * (g - BUFS + 1))  # TT#1 reads U,V
                    sync.wait_ge(dve_s3, G * (g - BUFS + 1))  # TT#2 reads U,W
                    sync.wait_ge(act_sq, G * (g - BUFS + 1))  # Square reads U
                b = g % BUFS
                sync.dma_start(out=U[b][:], in_=anc_r[g]).then_inc(SEM_U[b], 16)
                sync.dma_start(out=V[b][:], in_=pos_r[g]).then_inc(SEM_V[b], 16)
                sync.dma_start(out=W[b][:], in_=neg_r[g]).then_inc(SEM_W[b], 16)
            sync.wait_ge(act_sq, T)
            sync.wait_ge(act_s2, T)
            sync.wait_ge(dve_red, T)
            sync.dma_start(out=out_ap[0], in_=nu2[:]).then_inc(st_sem, 16)
            sync.dma_start(out=out_ap[1], in_=dotp[:]).then_inc(st_sem, 16)
            sync.dma_start(out=out_ap[2], in_=dotn[:]).then_inc(st_sem, 16)
            sync.wait_ge(st_sem, 48)

        @block.vector
        def _(vector):
            def red(t):
                # reduce for sub-tile t (issued one sub-tile late, so TT#2(t)
                # retired long before: no pipeline stall)
                vector.wait_ge(dve_s3, t + 1)
                nc.vector.tensor_reduce(
                    out=dotn[:, t : t + 1], in_=S3[t % 3][:], axis=AX, op=add
                ).then_inc(dve_red, 1)

            for g in range(NG):
                b = g % BUFS
                k = 16 * (g // BUFS + 1)
                vector.wait_ge(SEM_U[b], k)
                vector.wait_ge(SEM_V[b], k)
                for a in range(G):
                    t = g * G + a
                    if t >= 3:
                        vector.wait_ge(act_s2, t - 2)  # S2 slot consumed
                    nc.vector.tensor_tensor(
                        out=S2[t % 3][:], in0=U[b][:, a, :], in1=V[b][:, a, :], op=mult
                    ).then_inc(dve_s2, 1)
                    if a == 0:
                        vector.wait_ge(SEM_W[b], k)  # w loaded
                    if t >= 3:
                        vector.wait_ge(dve_red, t - 2)  # S3 slot reduce retired
                    nc.vector.tensor_tensor(
                        out=S3[t % 3][:], in0=U[b][:, a, :], in1=W[b][:, a, :], op=mult
                    ).then_inc(dve_s3, 1)
                    if t >= 1:
                        red(t - 1)
            red(T - 1)

        @block.scalar
        def _(scalar):
            def cp(t):
                # dotp reduce for sub-tile t (issued one sub-tile late)
                scalar.wait_ge(dve_s2, t + 1)  # product retired
                if t >= 2:
                    scalar.wait_ge(act_s2, t - 1)  # CPD slot writer retired
                nc.scalar.activation(
                    out=CPD[t % 2][:], in_=S2[t % 3][:], func=Cp,
                    accum_out=dotp[:, t : t + 1],
                ).then_inc(act_s2, 1)

            for g in range(NG):
                b = g % BUFS
                scalar.wait_ge(SEM_U[b], 16 * (g // BUFS + 1))  # u loaded
                for a in range(G):
                    t = g * G + a
                    if t >= 2:
                        scalar.wait_ge(act_sq, t - 1)  # SQD slot writer retired
                    nc.scalar.activation(
                        out=SQD[t % 2][:], in_=U[b][:, a, :], func=Sq,
                        accum_out=nu2[:, t : t + 1],
                    ).then_inc(act_sq, 1)
                    if t >= 1:
                        cp(t - 1)
            cp(T - 1)

    return nc


def kernel(embeddings, labels, pos_idx, neg_idx):
    global LAST_RESULT
    from concourse.bass_utils import run_bass_kernel_spmd

    emb = np.asarray(embeddings, dtype=np.float32).astype(np.float16)
    assert emb.shape == (N, D)
    pidx = np.asarray(pos_idx).astype(np.int64)
    nidx = np.asarray(neg_idx).astype(np.int64)

    in_maps = []
    for i in range(NCORES):
        sl = slice(i * RPC, (i + 1) * RPC)
        in_maps.append(
            {
                "anc": np.ascontiguousarray(emb[sl]),
                "pos": np.ascontiguousarray(emb[pidx[sl]]),
                "neg": np.ascontiguousarray(emb[nidx[sl]]),
            }
        )

    nc = _CACHE.get("nc")
    if nc is None:
        nc = _build_nc()
        _CACHE["nc"] = nc

    res = run_bass_kernel_spmd(nc, in_maps, list(range(NCORES)))
    LAST_RESULT = res

    # out[k] is [128, T]: row p, col t -> shard row t*128+p
    def decode(k):
        return np.concatenate(
            [res.results[i]["out"][k].T.ravel() for i in range(NCORES)]
        ).astype(np.float64)

    nu2 = decode(0)
    P = decode(1)
    Q = decode(2)

    norm = np.sqrt(nu2)
    den = np.maximum(norm, EPS)  # F.normalize clamp
    ahat2 = nu2 / (den * den)  # ||a_hat||^2 (==1 unless degenerate)

    def dist(idx, dot):
        S = ahat2 + ahat2[idx] - 2.0 * dot / (den * den[idx]) + D * EPS * EPS
        return np.sqrt(np.maximum(S, 0.0)) + EPS

    d_pos = dist(pidx, P)
    d_neg = dist(nidx, Q)
    pos_loss = d_pos * d_pos
    neg_loss = np.maximum(MARGIN - d_neg, EPS) ** 2
    total = pos_loss.sum() + neg_loss.sum()
    return np.array(total / (2.0 * N), dtype=np.float32)
